# revision 1
# baseline (speedup 1.0000x reference)
"""Distributed CalibreLoss kernel for 8 Trainium2 NeuronCores.

Computes (on device):
  full = concat(enc_a, enc_b)           # [32768, 512], row-sharded 8 ways
  labels = kmeans(full, 128, 10 iters)  # Lloyd, per-shard segment sums + AllReduce
  protos_a/b = segment_mean(proj_a/b)   # via onehot matmuls + AllReduce
  l_p = NTXent(protos_a, protos_b)      # replicated [256x128]
  support = segment_mean(enc_a)
  l_n = prototype CE(support, enc_b)    # per-shard + AllReduce of the sum
  loss = 0.5*l_p + 0.5*l_n

Sharding: core i holds rows [i*2048,(i+1)*2048) of both enc_a and enc_b
(plus matching projection rows), i.e. 4096 of the 32768 kmeans points.

Heavy matmuls run in float32r (fp32 rounded to 12-bit mantissa, 4x the
fp32 matmul rate at free-dim>=256). Labels never materialize as ints: the
one-hot assignment matrix is built directly by comparing scores to their
row max, and all segment sums / gathers are one-hot matmuls.

Measured: end-to-end relative error vs the fp32 reference ~1e-3
(kmeans label flips at fp32r precision dominate); modeled on-device
compute ~443 us/core (TimelineSim, collectives excluded); each [128,514]
AllReduce adds only a few us (measured marginal cost on hardware).
PE p-state is kept warm with filler matmuls: 11 per iteration gated on
the staged AllReduce payload (covering the serial tail where PE would
re-cool; cold PE runs matmuls at half rate, worth ~32 us over 10 Lloyd
iterations) plus 8 at kernel start to ramp during the first input DMAs.
"""

import sys

sys.path.insert(0, '/opt/trn_rl_repo')

import numpy as np

N_CORES = 8
B = 16384          # rows per encodings tensor
DE = 512           # encoding dim
DP = 128           # projection dim
C = 128            # n_clusters
N_ITERS = 10
TEMP = 0.5
PER = B // N_CORES          # 2048 a-rows (and b-rows) per core
NBLK = 2 * PER // 128       # 32 point-blocks of 128 per core (0-15 a, 16-31 b)
NTILE = NBLK // 4           # 8 point-tiles of 512 per core
KCH = DE // 128             # 4 feature chunks

_PROG = None


def _build(n_iters=N_ITERS, do_final=True, repeat=1, use_cc=True, n_devices=N_CORES, probes=()):
    import concourse.bacc as bacc
    import concourse.mybir as mybir
    import concourse.tile as tile
    from concourse.masks import make_identity

    dt = mybir.dt
    f32 = dt.float32
    f32r = dt.float32r
    Alu = mybir.AluOpType
    Act = mybir.ActivationFunctionType
    Ax = mybir.AxisListType

    nc = bacc.Bacc('TRN2', target_bir_lowering=False, num_devices=n_devices)
    xa = nc.dram_tensor("xa", [PER, DE], f32, kind="ExternalInput")
    xb = nc.dram_tensor("xb", [PER, DE], f32, kind="ExternalInput")
    pa = nc.dram_tensor("pa", [PER, DP], f32, kind="ExternalInput")
    pb = nc.dram_tensor("pb", [PER, DP], f32, kind="ExternalInput")
    c0 = nc.dram_tensor("c0", [C, DE], f32, kind="ExternalInput")
    out = nc.dram_tensor("loss", [1, 1], f32, kind="ExternalOutput")
    RG = [list(range(N_CORES))]

    with tile.TileContext(nc) as tc, \
         tc.tile_pool(name="data", bufs=1) as data, \
         tc.tile_pool(name="sc", bufs=2) as sc, \
         tc.tile_pool(name="small", bufs=2) as small, \
         tc.tile_pool(name="ld", bufs=3) as ldp, \
         tc.tile_pool(name="ps_score", bufs=3, space="PSUM") as ps_score, \
         tc.tile_pool(name="ps_trans", bufs=3, space="PSUM") as ps_trans, \
         tc.tile_pool(name="ps_seg", bufs=1, space="PSUM") as ps_seg, \
         tc.tile_pool(name="ps_cnt", bufs=1, space="PSUM") as ps_cnt, \
         tc.tile_pool(name="dram", bufs=2, space="DRAM") as drp:

        # ---------- persistent SBUF ----------
        eye = data.tile([128, 128], f32, name="eye")
        make_identity(nc, eye[:])
        eye_r = data.tile([128, 128], f32r, name="eye_r")
        nc.vector.tensor_copy(out=eye_r[:], in_=eye[:])
        ones_f = data.tile([128, 2], f32, name="ones_f")
        nc.vector.memset(ones_f[:], 1.0)
        ones_r = data.tile([128, 2], f32r, name="ones_r")
        nc.vector.tensor_copy(out=ones_r[:], in_=ones_f[:])
        xT_all = data.tile([128, KCH * NBLK * 128], f32r, name="xT_all")
        xT = [xT_all[:, k * NBLK * 128:(k + 1) * NBLK * 128] for k in range(KCH)]
        xpm_all = data.tile([128, NBLK * DE], f32r, name="xpm_all")
        xpm = [xpm_all[:, b * DE:(b + 1) * DE] for b in range(NBLK)]
        oh = [data.tile([128, C], f32r, name=f"oh{b}") for b in range(NBLK)]
        if probes:
            for _b in range(NBLK):
                nc.gpsimd.memset(oh[_b][:], 0.0)
        cents = data.tile([C, DE], f32, name="cents")
        centsT = [data.tile([128, C], f32r, name=f"centsT{k}") for k in range(KCH)]
        negcc = data.tile([C, 1], f32, name="negcc")
        cc_pos = data.tile([C, 1], f32, name="cc_pos")
        sq_scratch = data.tile([128, DE], f32, name="sq_scratch")
        contrib_all = data.tile([128, 16], f32, name="contrib_all")

        for _rep in range(repeat):
            # ---------- load + transpose setup ----------
            nc.sync.dma_start(out=cents[:], in_=c0[:])
            if "nowarm" not in probes:
                # ramp the PE p-state while the first input DMAs land
                warm0_ps = ps_cnt.tile([C, 512], f32, name="warm0_ps", tag="cnt")
                for w0 in range(8):
                    nc.tensor.matmul(warm0_ps[:, 0:128], lhsT=eye[:], rhs=eye[:],
                                     start=True, stop=True)
            for g in range(NBLK // 4):  # 4 blocks per load group
                b0 = g * 4
                src = xa if b0 < 16 else xb
                r0 = (b0 % 16) * 128
                blk4 = ldp.tile([128, 4, DE], f32, name="blk4", tag="blk4")
                nc.sync.dma_start(
                    out=blk4[:],
                    in_=src[r0:r0 + 512, :].rearrange("(b p) f -> p b f", p=128))
                nc.scalar.copy(
                    out=xpm_all[:, b0 * DE:(b0 + 4) * DE].rearrange(
                        "p (b f) -> p b f", b=4),
                    in_=blk4[:])
                for bi in range(4):
                    b = b0 + bi
                    tp = ps_trans.tile([128, 512], f32, name="tp", tag="tp")
                    for k in range(KCH):
                        nc.tensor.transpose(out=tp[:, k * 128:(k + 1) * 128],
                                            in_=blk4[:, bi, k * 128:(k + 1) * 128],
                                            identity=eye[:])
                    dst = xT_all[:].rearrange(
                        "p (k n) -> p k n", k=KCH)[:, :, b * 128:(b + 1) * 128]
                    nc.vector.tensor_copy(out=dst, in_=tp[:])

            def prep_cents():
                # negcc = -0.5 * sum(cents^2, axis=1); centsT = cents.T rounded
                nc.scalar.activation(out=sq_scratch[:], in_=cents[:],
                                     func=Act.Square, scale=1.0, accum_out=cc_pos[:])
                nc.vector.tensor_scalar_mul(negcc[:], cc_pos[:], -0.5)
                tp = ps_trans.tile([128, 512], f32, name="tpc", tag="tp")
                for k in range(KCH):
                    nc.tensor.transpose(out=tp[:, k * 128:(k + 1) * 128],
                                        in_=cents[:, k * 128:(k + 1) * 128],
                                        identity=eye[:])
                for k in range(KCH):
                    nc.vector.tensor_copy(out=centsT[k][:],
                                          in_=tp[:, k * 128:(k + 1) * 128])

            prep_cents()

            # ---------- Lloyd iterations + final assignment ----------
            for t in range(n_iters + 1):
                last = (t == n_iters)
                if not last:
                    seg_ps = ps_seg.tile([C, DE], f32, name="seg_ps", tag="seg")
                    cnt_ps = ps_cnt.tile([C, 2], f32, name="cnt_ps", tag="cnt")
                for ti in range(NTILE):
                    sc_ps = ps_score.tile([C, 512], f32, name="sc_ps", tag="sc")
                    for k in range(KCH):
                        nc.tensor.matmul(sc_ps[:], lhsT=centsT[k][:],
                                         rhs=xT[k][:, ti * 512:(ti + 1) * 512],
                                         start=(k == 0), stop=(k == KCH - 1))
                    sc_sb = sc.tile([C, 512], f32r, name="sc_sb", tag="scsb")
                    nc.scalar.activation(out=sc_sb[:], in_=sc_ps[:], func=Act.Identity,
                                         bias=negcc[:, 0:1], scale=1.0)
                    tr_psr = ps_trans.tile([128, 512], f32r, name="tr_psr", tag="tp")
                    for s in range(4):
                        nc.tensor.transpose(out=tr_psr[:, s * 128:(s + 1) * 128],
                                            in_=sc_sb[:, s * 128:(s + 1) * 128],
                                            identity=eye_r[:])
                    tr_ps = tr_psr[:].bitcast(f32)
                    rm = small.tile([128, 4], f32, name="rm", tag="rm")
                    if "normax" not in probes:
                        nc.vector.tensor_reduce(out=rm[:],
                                                in_=tr_ps.rearrange("p (b c) -> p b c", b=4),
                                                axis=Ax.X, op=Alu.max)
                    for s in range(4):
                        b = ti * 4 + s
                        if "noiseq" not in probes:
                            nc.vector.tensor_scalar(
                                out=oh[b][:], in0=tr_ps[:, s * 128:(s + 1) * 128],
                                scalar1=rm[:, s:s + 1], scalar2=None, op0=Alu.is_equal)
                        if not last:
                            if "noseg" not in probes:
                                nc.tensor.matmul(seg_ps[:], lhsT=oh[b][:], rhs=xpm[b][:],
                                                 start=(b == 0), stop=(b == NBLK - 1))
                            if "nocnt" not in probes:
                                nc.tensor.matmul(cnt_ps[:, 0:2], lhsT=oh[b][:], rhs=ones_r[:],
                                                 start=(b == 0), stop=(b == NBLK - 1))
                if last:
                    break
                if "notail" in probes:
                    continue
                stage = sc.tile([C, DE + 2], f32, name="stage", tag="stage")
                if "noseg" in probes:
                    nc.scalar.activation(out=stage[:, 0:DE], in_=sq_scratch[:],
                                         func=Act.Identity, bias=0.0, scale=1.0)
                    nc.scalar.copy(out=stage[:, DE:DE + 2], in_=sq_scratch[:, 0:2])
                else:
                    nc.scalar.copy(out=stage[:, 0:DE], in_=seg_ps[:])
                    nc.scalar.copy(out=stage[:, DE:DE + 2],
                                   in_=cnt_ps[:, 0:2] if "nocnt" not in probes
                                   else sq_scratch[:, 0:2])
                arin = drp.tile([C, DE + 2], f32, name="arin", tag="arin")
                arout = drp.tile([C, DE + 2], f32, name="arout", tag="arout",
                                 addr_space="Shared" if use_cc else "Local")
                nc.sync.dma_start(out=arin[:], in_=stage[:])
                if "nowarm" not in probes:
                    # keep the PE p-state warm through the AllReduce tail:
                    # fp32 filler matmuls gated on `stage` land exactly in the
                    # serial window where PE would otherwise idle and re-cool.
                    warm_ps = ps_cnt.tile([C, 512], f32, name="warm_ps", tag="cnt")
                    for w in range(11):
                        nc.tensor.matmul(warm_ps[:], lhsT=stage[:, 0:128],
                                         rhs=stage[:, 0:512], start=True, stop=True)
                if use_cc:
                    nc.gpsimd.collective_compute("AllReduce", Alu.add, replica_groups=RG,
                                                 ins=[arin.opt()], outs=[arout.opt()])
                else:
                    nc.sync.dma_start(out=arout[:], in_=arin[:])
                gst = sc.tile([C, DE + 2], f32, name="gst", tag="stage")
                nc.sync.dma_start(out=gst[:], in_=arout[:])
                cnt_clip = small.tile([C, 1], f32, name="cnt_clip", tag="cc1")
                nc.vector.tensor_scalar_max(cnt_clip[:], gst[:, DE:DE + 1], 1.0)
                cnt_rec = small.tile([C, 1], f32, name="cnt_rec", tag="cc2")
                nc.vector.reciprocal(cnt_rec[:], cnt_clip[:])
                mask = small.tile([C, 1], f32, name="mask", tag="cc3")
                nc.vector.tensor_scalar(mask[:], gst[:, DE:DE + 1], 0.0, None,
                                        op0=Alu.is_gt)
                delta = sc.tile([C, DE], f32, name="delta", tag="scsb")
                # delta = mean - cents = gst*rec - cents ; cents += mask*delta
                nc.vector.scalar_tensor_tensor(out=delta[:], in0=gst[:, 0:DE],
                                               scalar=cnt_rec[:, 0:1], in1=cents[:],
                                               op0=Alu.mult, op1=Alu.subtract)
                nc.vector.scalar_tensor_tensor(out=cents[:], in0=delta[:],
                                               scalar=mask[:, 0:1], in1=cents[:],
                                               op0=Alu.mult, op1=Alu.add)
                prep_cents()

            if do_final:
                # ---------- final segment sums (protos a/b, support, counts) ----------
                pa_ps = ps_score.tile([C, 512], f32, name="pa_ps", tag="sc")
                pb_ps = ps_trans.tile([128, 512], f32, name="pb_ps", tag="tp")
                sup_ps = ps_seg.tile([C, DE], f32, name="sup_ps", tag="seg")
                ca_ps = ps_cnt.tile([C, 2], f32, name="ca_ps", tag="cnt")
                cb_ps = ps_cnt.tile([C, 2], f32, name="cb_ps", tag="cnt")
                for b in range(16):
                    prj = ldp.tile([128, DP], f32, name="prj", tag="prj")
                    nc.sync.dma_start(out=prj[:], in_=pa[(b % 16) * 128:(b % 16 + 1) * 128, :])
                    prjr = ldp.tile([128, DP], f32r, name="prjr", tag="prjr")
                    nc.vector.tensor_copy(out=prjr[:], in_=prj[:])
                    nc.tensor.matmul(pa_ps[:, 0:DP], lhsT=oh[b][:], rhs=prjr[:],
                                     start=(b == 0), stop=(b == 15))
                    nc.tensor.matmul(sup_ps[:], lhsT=oh[b][:], rhs=xpm[b][:],
                                     start=(b == 0), stop=(b == 15))
                    nc.tensor.matmul(ca_ps[:, 0:2], lhsT=oh[b][:], rhs=ones_r[:],
                                     start=(b == 0), stop=(b == 15))
                for b in range(16, NBLK):
                    prj = ldp.tile([128, DP], f32, name="prj", tag="prj")
                    nc.sync.dma_start(out=prj[:], in_=pb[(b % 16) * 128:(b % 16 + 1) * 128, :])
                    prjr = ldp.tile([128, DP], f32r, name="prjr", tag="prjr")
                    nc.vector.tensor_copy(out=prjr[:], in_=prj[:])
                    nc.tensor.matmul(pb_ps[:, 0:DP], lhsT=oh[b][:], rhs=prjr[:],
                                     start=(b == 16), stop=(b == NBLK - 1))
                    nc.tensor.matmul(cb_ps[:, 0:2], lhsT=oh[b][:], rhs=ones_r[:],
                                     start=(b == 16), stop=(b == NBLK - 1))
                W = 2 * DP + DE + 2  # 770
                stage2 = sc.tile([C, W], f32, name="stage2", tag="stage")
                nc.scalar.copy(out=stage2[:, 0:DP], in_=pa_ps[:, 0:DP])
                nc.scalar.copy(out=stage2[:, DP:2 * DP], in_=pb_ps[:, 0:DP])
                nc.scalar.copy(out=stage2[:, 2 * DP:2 * DP + DE], in_=sup_ps[:])
                nc.scalar.copy(out=stage2[:, W - 2:W - 1], in_=ca_ps[:, 0:1])
                nc.scalar.copy(out=stage2[:, W - 1:W], in_=cb_ps[:, 0:1])
                ar1i = drp.tile([C, W], f32, name="ar1i", tag="ar1i")
                ar1o = drp.tile([C, W], f32, name="ar1o", tag="ar1o", addr_space="Shared" if use_cc else "Local")
                nc.sync.dma_start(out=ar1i[:], in_=stage2[:])
                if use_cc:
                    nc.gpsimd.collective_compute("AllReduce", Alu.add, replica_groups=RG,
                                                 ins=[ar1i.opt()], outs=[ar1o.opt()])
                else:
                    nc.sync.dma_start(out=ar1o[:], in_=ar1i[:])
                gs1 = sc.tile([C, W], f32, name="gs1", tag="stage")
                nc.sync.dma_start(out=gs1[:], in_=ar1o[:])

                # ---------- means ----------
                ca_clip = small.tile([C, 1], f32, name="ca_clip", tag="cc1")
                nc.vector.tensor_scalar_max(ca_clip[:], gs1[:, W - 2:W - 1], 1.0)
                ra = small.tile([C, 1], f32, name="ra", tag="cc2")
                nc.vector.reciprocal(ra[:], ca_clip[:])
                cb_clip = small.tile([C, 1], f32, name="cb_clip", tag="cc3")
                nc.vector.tensor_scalar_max(cb_clip[:], gs1[:, W - 1:W], 1.0)
                rb = small.tile([C, 1], f32, name="rb", tag="cc4")
                nc.vector.reciprocal(rb[:], cb_clip[:])
                pam = sc.tile([C, DP], f32, name="pam", tag="pam")
                nc.vector.tensor_scalar_mul(pam[:], gs1[:, 0:DP], ra[:, 0:1])
                pbm = sc.tile([C, DP], f32, name="pbm", tag="pbm")
                nc.vector.tensor_scalar_mul(pbm[:], gs1[:, DP:2 * DP], rb[:, 0:1])
                supm = sc.tile([C, DE], f32, name="supm", tag="supm")
                nc.vector.tensor_scalar_mul(supm[:], gs1[:, 2 * DP:2 * DP + DE], ra[:, 0:1])

                # ---------- NTXent on prototypes ----------
                def normalize(p_mean, tag):
                    n2 = small.tile([C, 1], f32, name=f"n2{tag}", tag=f"n2{tag}")
                    nc.scalar.activation(out=sq_scratch[:, 0:DP], in_=p_mean[:],
                                         func=Act.Square, scale=1.0, accum_out=n2[:])
                    nrm = small.tile([C, 1], f32, name=f"nrm{tag}", tag=f"nrm{tag}")
                    nc.scalar.sqrt(nrm[:], n2[:])
                    nc.vector.tensor_scalar_max(nrm[:], nrm[:], 1e-8)
                    rn = small.tile([C, 1], f32, name=f"rn{tag}", tag=f"rn{tag}")
                    nc.vector.reciprocal(rn[:], nrm[:])
                    z = sc.tile([C, DP], f32, name=f"z{tag}", tag=f"z{tag}")
                    nc.vector.tensor_scalar_mul(z[:], p_mean[:], rn[:, 0:1])
                    return z

                za = normalize(pam, "a")
                zb = normalize(pbm, "b")
                zT = sc.tile([128, 256], f32, name="zT", tag="scsb")
                tpz = ps_trans.tile([128, 512], f32, name="tpz", tag="tp")
                nc.tensor.transpose(out=tpz[:, 0:128], in_=za[:], identity=eye[:])
                nc.tensor.transpose(out=tpz[:, 128:256], in_=zb[:], identity=eye[:])
                nc.scalar.copy(out=zT[:, 0:128], in_=tpz[:, 0:128])
                nc.scalar.copy(out=zT[:, 128:256], in_=tpz[:, 128:256])
                eye9 = sc.tile([128, 128], f32, name="eye9", tag="junk2")
                nc.vector.tensor_scalar_mul(eye9[:], eye[:], 1e9)

                lp_vec = small.tile([C, 1], f32, name="lp_vec", tag="lp_vec")
                for half, diag_lo in ((0, True), (1, False)):
                    # half 0: rows = a-protos (diag in cols 0:128)... careful:
                    # a-rows have self-sim in cols 0:128, positives in cols 128:256
                    # b-rows have self-sim in cols 128:256, positives in cols 0:128
                    sim_ps = ps_score.tile([C, 512], f32, name="sim_ps", tag="sc")
                    nc.tensor.matmul(sim_ps[:, 0:256],
                                     lhsT=zT[:, half * 128:(half + 1) * 128],
                                     rhs=zT[:], start=True, stop=True)
                    sim_sb = sc.tile([128, 256], f32, name="sim_sb", tag="sim_sb")
                    dcol = 0 if half == 0 else 128
                    pcol = 128 - dcol
                    nc.vector.tensor_sub(sim_sb[:, dcol:dcol + 128],
                                         sim_ps[:, dcol:dcol + 128], eye9[:])
                    nc.scalar.copy(out=sim_sb[:, pcol:pcol + 128],
                                   in_=sim_ps[:, pcol:pcol + 128])
                    m = small.tile([C, 1], f32, name="m", tag="m")
                    nc.vector.tensor_reduce(out=m[:], in_=sim_sb[:], axis=Ax.X, op=Alu.max)
                    negm2 = small.tile([C, 1], f32, name="negm2", tag="negm2")
                    nc.vector.tensor_scalar_mul(negm2[:], m[:], -2.0)
                    expj = sc.tile([128, 256], f32, name="expj", tag="expj")
                    se = small.tile([C, 1], f32, name="se", tag="se")
                    nc.scalar.activation(out=expj[:], in_=sim_sb[:], func=Act.Exp,
                                         bias=negm2[:, 0:1], scale=2.0, accum_out=se[:])
                    pk = small.tile([C, 1], f32, name="pk", tag="pk")
                    junk = sc.tile([128, 128], f32, name="junk2", tag="junk2")
                    nc.vector.scalar_tensor_tensor(out=junk[:], in0=sim_sb[:, pcol:pcol + 128],
                                                   scalar=1.0, in1=eye[:], op0=Alu.mult,
                                                   op1=Alu.mult, accum_out=pk[:])
                    lse = small.tile([C, 1], f32, name="lse", tag="lse")
                    nc.scalar.activation(out=lse[:], in_=se[:], func=Act.Ln)
                    ctr = small.tile([C, 1], f32, name="ctr", tag="ctr")
                    nc.vector.scalar_tensor_tensor(out=ctr[:], in0=pk[:], scalar=2.0,
                                                   in1=negm2[:], op0=Alu.mult, op1=Alu.add)
                    nc.vector.tensor_sub(ctr[:], ctr[:], lse[:])
                    if half == 0:
                        nc.vector.tensor_copy(out=lp_vec[:], in_=ctr[:])
                    else:
                        nc.vector.tensor_add(lp_vec[:], lp_vec[:], ctr[:])

                # ---------- prototype CE loss on encodings_b ----------
                negss = data.tile([C, 1], f32, name="negss")
                ss_pos = data.tile([C, 1], f32, name="ss_pos")
                nc.scalar.activation(out=sq_scratch[:], in_=supm[:],
                                     func=Act.Square, scale=1.0, accum_out=ss_pos[:])
                nc.vector.tensor_scalar_mul(negss[:], ss_pos[:], -1.0)
                supT = [data.tile([128, C], f32r, name=f"supT{k}") for k in range(KCH)]
                tps = ps_trans.tile([128, 512], f32, name="tps", tag="tp")
                for k in range(KCH):
                    nc.tensor.transpose(out=tps[:, k * 128:(k + 1) * 128],
                                        in_=supm[:, k * 128:(k + 1) * 128], identity=eye[:])
                for k in range(KCH):
                    nc.vector.tensor_copy(out=supT[k][:], in_=tps[:, k * 128:(k + 1) * 128])
                for ti in range(4, 8):  # b-point tiles
                    ln_ps = ps_score.tile([C, 512], f32, name="ln_ps", tag="sc")
                    for k in range(KCH):
                        nc.tensor.matmul(ln_ps[:], lhsT=supT[k][:],
                                         rhs=xT[k][:, ti * 512:(ti + 1) * 512],
                                         start=(k == 0), stop=(k == KCH - 1))
                    ln_sb = sc.tile([C, 512], f32, name="ln_sb", tag="scsb")
                    nc.scalar.activation(out=ln_sb[:], in_=ln_ps[:], func=Act.Identity,
                                         bias=negss[:, 0:1], scale=2.0)
                    tr2 = ps_trans.tile([128, 512], f32, name="tr2", tag="tp")
                    for s in range(4):
                        nc.tensor.transpose(out=tr2[:, s * 128:(s + 1) * 128],
                                            in_=ln_sb[:, s * 128:(s + 1) * 128],
                                            identity=eye[:])
                    rm4 = small.tile([128, 4], f32, name="rm4", tag="rm")
                    nc.vector.tensor_reduce(out=rm4[:],
                                            in_=tr2[:].rearrange("p (b c) -> p b c", b=4),
                                            axis=Ax.X, op=Alu.max)
                    nrm4 = small.tile([128, 4], f32, name="nrm4", tag="nrm4")
                    nc.vector.tensor_scalar_mul(nrm4[:], rm4[:], -1.0)
                    se4 = small.tile([128, 4], f32, name="se4", tag="se4")
                    pk4 = small.tile([128, 4], f32, name="pk4", tag="pk4")
                    expj2 = sc.tile([128, 512], f32, name="expj2", tag="expj")
                    junk3 = sc.tile([128, 128], f32, name="junk3", tag="junk2")
                    for s in range(4):
                        b = ti * 4 + s
                        nc.scalar.activation(out=expj2[:, s * 128:(s + 1) * 128],
                                             in_=tr2[:, s * 128:(s + 1) * 128], func=Act.Exp,
                                             bias=nrm4[:, s:s + 1], scale=1.0,
                                             accum_out=se4[:, s:s + 1])
                        nc.vector.scalar_tensor_tensor(
                            out=junk3[:], in0=tr2[:, s * 128:(s + 1) * 128], scalar=1.0,
                            in1=oh[b][:].bitcast(f32), op0=Alu.mult, op1=Alu.mult,
                            accum_out=pk4[:, s:s + 1])
                    lse4 = small.tile([128, 4], f32, name="lse4", tag="lse4")
                    nc.scalar.activation(out=lse4[:], in_=se4[:], func=Act.Ln)
                    cslice = contrib_all[:, (ti - 4) * 4:(ti - 3) * 4]
                    nc.vector.tensor_sub(cslice, pk4[:], rm4[:])
                    nc.vector.tensor_sub(cslice, cslice, lse4[:])
                ln_vec = small.tile([128, 1], f32, name="ln_vec", tag="ln_vec")
                nc.vector.tensor_reduce(out=ln_vec[:], in_=contrib_all[:], axis=Ax.X,
                                        op=Alu.add)

                # ---------- reduce over partitions, AllReduce l_n, combine ----------
                red_in = small.tile([128, 2], f32, name="red_in", tag="red")
                nc.vector.tensor_copy(out=red_in[:, 0:1], in_=ln_vec[:])
                nc.vector.tensor_copy(out=red_in[:, 1:2], in_=lp_vec[:])
                red_ps = ps_seg.tile([1, 2], f32, name="red_ps", tag="seg")
                nc.tensor.matmul(red_ps[:], lhsT=ones_f[:, 0:1], rhs=red_in[:],
                                 start=True, stop=True)
                red_sb = small.tile([1, 2], f32, name="red_sb", tag="red_sb")
                nc.scalar.copy(out=red_sb[:], in_=red_ps[:])
                # fold the loss combine BEFORE the AllReduce: every core
                # contributes -0.5/B * ln_partial + (lp term)/n_cores, so the
                # AR output IS the loss and DMAs straight to the output.
                lp_t = small.tile([1, 1], f32, name="lp_t", tag="lp_t")
                nc.vector.tensor_scalar_mul(lp_t[:], red_sb[0:1, 1:2],
                                            -0.5 / (2 * C) / N_CORES)
                loss_sb = small.tile([1, 1], f32, name="loss_sb", tag="loss_sb")
                nc.vector.scalar_tensor_tensor(out=loss_sb[:], in0=red_sb[0:1, 0:1],
                                               scalar=-0.5 / B, in1=lp_t[:],
                                               op0=Alu.mult, op1=Alu.add)
                ar3i = drp.tile([1, 1], f32, name="ar3i", tag="ar3i")
                ar3o = drp.tile([1, 1], f32, name="ar3o", tag="ar3o", addr_space="Shared" if use_cc else "Local")
                nc.sync.dma_start(out=ar3i[:], in_=loss_sb[:])
                if use_cc:
                    nc.gpsimd.collective_compute("AllReduce", Alu.add, replica_groups=RG,
                                                 ins=[ar3i.opt()], outs=[ar3o.opt()])
                else:
                    nc.sync.dma_start(out=ar3o[:], in_=ar3i[:])
                nc.sync.dma_start(out=out[:], in_=ar3o[:])


            else:
                nodum = small.tile([1, 1], f32, name="nodum", tag="loss_sb")
                nc.vector.tensor_copy(out=nodum[:], in_=oh[0][0:1, 0:1].bitcast(f32))
                nc.sync.dma_start(out=out[:], in_=nodum[:])

    nc.compile()
    return nc


def kernel(encodings_a, encodings_b, projections_a, projections_b, n_clusters):
    assert int(n_clusters) == C
    ea = np.ascontiguousarray(np.asarray(encodings_a, dtype=np.float32))
    eb = np.ascontiguousarray(np.asarray(encodings_b, dtype=np.float32))
    pra = np.ascontiguousarray(np.asarray(projections_a, dtype=np.float32))
    prb = np.ascontiguousarray(np.asarray(projections_b, dtype=np.float32))
    global _PROG
    if _PROG is None:
        _PROG = _build()
    nc = _PROG
    c0 = np.ascontiguousarray(ea[:C])
    in_maps = []
    for i in range(N_CORES):
        sl = slice(i * PER, (i + 1) * PER)
        in_maps.append({
            "xa": ea[sl], "xb": eb[sl], "pa": pra[sl], "pb": prb[sl], "c0": c0,
        })
    from concourse.bass_utils import run_bass_kernel_spmd
    res = run_bass_kernel_spmd(nc, in_maps, core_ids=list(range(N_CORES)))
    loss = np.asarray(res.results[0]["loss"], dtype=np.float32).reshape(())
    return loss

